# revision 20
# baseline (speedup 1.0000x reference)
"""Trainium2 Bass kernel for a ReActNet-style binary BasicBlock.

Reference math per block (twice, with different weights):
    s   = sign(x + b_in)                      # +-1
    c   = conv3x3(s, mean|w| * sign(w))       # binarized conv, pad=1
    y   = x + ALPHA * c                       # residual
    y   = prelu(y + b_mid, a) + b_out

Key facts exploited:
  * matmul inputs are exactly +-1 -> bf16 matmuls are EXACT (integer sums
    accumulated in fp32 PSUM).
  * per-output-channel weight scale factors out:  conv(s, scale*sign(w)) =
    scale .* conv(s, sign(w)).
  * residual rides through PSUM via an fp32 matmul with diag(1/(ALPHA*scale)):
        T = binconv(s) + x / as           (as = ALPHA*scale, per channel)
    then prelu(x + as*binconv + b, a) = Prelu-activation(T) with
    per-partition scale=as, bias=b, alpha=a  -- a single ScalarE op reading
    PSUM directly.  (prelu positive homogeneity: as > 0.)

Layout: NCHW, channels (64) on partitions; 2 images stacked per 128
partitions (img even -> partitions 0-63, img odd -> 64-127).  Conv matmuls
run as two concurrent 64x64 PE tiles on the array diagonal.  Spatial strips
of R=16 output rows, width padded to 114 with zero columns held in the
sign tiles; conv taps are constant flat-offset shifts.

Sharding: pure data parallel, batch 32 -> 4 images x 8 cores, weights
replicated, no collectives.
"""

import numpy as np
from contextlib import ExitStack

import concourse.bass as bass
import concourse.tile as tile
from concourse import mybir
from concourse import bacc
from concourse.bass_utils import run_bass_kernel_spmd
from concourse.masks import make_identity

B, C, H, W = 32, 64, 112, 112
ALPHA = 0.25
NCORES = 8
BL = B // NCORES          # images per core
WP = W + 2                # padded width
R = 16                    # output rows per strip
NSTRIPS = H // R

F32 = mybir.dt.float32
BF16 = mybir.dt.bfloat16

WVEC_NAMES = ["b11", "b12", "b13", "b21", "b22", "b23", "a1", "a2"]

CONV_SCHEME = "diag2"  # "quad" | "diag2"


def _bcast_ap(dram_ap, reps=2):
    """Source AP replicating a DRAM tensor across partition groups."""
    return bass.AP(
        tensor=dram_ap.tensor,
        offset=dram_ap.offset,
        ap=[[0, reps]] + [list(d) for d in dram_ap.ap],
    )


def _row_chunks(lo, hi, step=4):
    r = lo
    while r < hi:
        yield r, min(step, hi - r)
        r += step


def build_program(bl=BL):
    """Build the Bass program for one core processing `bl` images."""
    nc = bacc.Bacc("TRN2", target_bir_lowering=False, debug=False)

    x_d = nc.dram_tensor("x", [bl, C, H, W], F32, kind="ExternalInput").ap()
    w3_d = nc.dram_tensor("w3", [C, C, 3, 3], F32, kind="ExternalInput").ap()
    wpw_d = nc.dram_tensor("w_pw", [C, C, 3, 3], F32, kind="ExternalInput").ap()
    vec_d = {
        n: nc.dram_tensor(n, [C], F32, kind="ExternalInput").ap()
        for n in WVEC_NAMES
    }
    out_d = nc.dram_tensor("out", [bl, C, H, W], F32, kind="ExternalOutput").ap()

    with tile.TileContext(nc) as tc:
        _kernel_body(tc, out_d, x_d, w3_d, wpw_d, vec_d, bl)

    nc.compile()
    return nc


def _prep_conv_consts(nc, const, wdram, name):
    """Per-conv constants: binarized-transposed weights, as=ALPHA*mean|w|,
    diag(1/as) for the residual matmul.  Everything replicated on both
    partition halves."""
    # natural layout [co, ci*3*3] duplicated -> per-channel scale
    wn = const.tile([128, C * 9], F32, name=f"wn_{name}")
    nc.sync.dma_start(out=wn, in_=_bcast_ap(wdram.rearrange("a b c d -> a (b c d)")))
    wabs = const.tile([128, C * 9], F32, name=f"wabs_{name}")
    asum = const.tile([128, 1], F32, name=f"asum_{name}")
    nc.scalar.activation(
        out=wabs, in_=wn, func=mybir.ActivationFunctionType.Abs, accum_out=asum
    )
    asc = const.tile([128, 1], F32, name=f"asc_{name}")  # ALPHA * mean|w|
    nc.vector.tensor_scalar_mul(asc, asum, ALPHA / (C * 9))
    inv_asc = const.tile([128, 1], F32, name=f"iasc_{name}")
    nc.vector.reciprocal(inv_asc, asc)

    # gathered+transposed weights [ci(+dup), co, tap], then binarize to bf16
    wg = const.tile([128, C, 9], F32, name=f"wg_{name}")
    src = bass.AP(
        tensor=wdram.tensor,
        offset=wdram.offset,
        ap=[[9, C], [C * 9, C], [1, 9]],
    )
    for rep in range(2):
        nc.sync.dma_start(out=wg[64 * rep : 64 * rep + 64, :, :], in_=src)
    wsign = const.tile([128, C, 9], BF16, name=f"ws_{name}")
    nc.scalar.activation(out=wsign, in_=wg, func=mybir.ActivationFunctionType.Sign)

    # residual injector: diag(1/as) fp32, per partition half
    ident = const.tile([128, C], F32, name=f"id_{name}")
    make_identity(nc, ident[0:64, :])
    make_identity(nc, ident[64:128, :])
    nc.vector.tensor_scalar_mul(ident, ident, inv_asc)
    return wsign, asc, ident


def _conv_bankset(nc, pt, w, ident, s_t, soff, res_t, roff, n, asc, bias_mid,
                  alpha, out_ap, ev):
    """One 2-bank PSUM set covering both images of the pair.

    Four concurrent 64x64 PE quadrants:
      (row 0, col 0): imgA subset1 -> bank0[0:64]
      (row 64, col 64): imgB subset1 -> bank0[64:128]
      (row 0, col 64): imgA subset2 -> bank1[64:128]   (crossed)
      (row 64, col 0): imgB subset2 -> bank1[0:64]     (crossed)
    subset1 = {residual-injector matmul, taps 0-3}; subset2 = {taps 4-8}.
    Eviction: u = as*bank0 + bias (ACT, full width); u += as*bank1 via two
    half-width cross-partition STTs (DVE); out = Prelu(u) (ACT).
    """
    lo, hi = slice(0, 64), slice(64, 128)
    if CONV_SCHEME == "quad":
        quads = [(lo, lo, 0), (hi, hi, 0), (lo, hi, 1), (hi, lo, 1)]
    else:  # diag2: only diagonal quadrants, banks aligned
        quads = [(lo, lo, 0), (hi, hi, 0), (lo, lo, 1), (hi, hi, 1)]
    slots = (
        [("id", None), ("tap", 0), ("tap", 1), ("tap", 2), ("tap", 3)],
        [("tap", 4), ("tap", 5), ("tap", 6), ("tap", 7), ("tap", 8)],
    )
    for s in range(5):
        for rsl, osl, sub in quads:
            kind, t = slots[sub][s]
            boff = 0 if sub == 0 else 512
            out = pt[osl, boff : boff + n]
            if kind == "id":
                nc.tensor.matmul(
                    out, ident[rsl, :], res_t[rsl, roff : roff + n],
                    start=True, stop=False, skip_group_check=True,
                )
            else:
                nc.tensor.matmul(
                    out, w[rsl, :, t], s_t[rsl, soff(t) : soff(t) + n],
                    start=(s == 0), stop=(s == 4), skip_group_check=True,
                )
    nc.scalar.activation(
        out=ev[:, :n], in_=pt[:, 0:n],
        func=mybir.ActivationFunctionType.Identity, bias=bias_mid, scale=asc,
    )
    if CONV_SCHEME == "quad":
        nc.vector.scalar_tensor_tensor(
            out=ev[0:64, :n], in0=pt[64:128, 512 : 512 + n], scalar=asc[0:64],
            in1=ev[0:64, :n], op0=mybir.AluOpType.mult, op1=mybir.AluOpType.add,
        )
        nc.vector.scalar_tensor_tensor(
            out=ev[64:128, :n], in0=pt[0:64, 512 : 512 + n],
            scalar=asc[64:128], in1=ev[64:128, :n],
            op0=mybir.AluOpType.mult, op1=mybir.AluOpType.add,
        )
    else:
        nc.vector.scalar_tensor_tensor(
            out=ev[:, :n], in0=pt[:, 512 : 512 + n], scalar=asc,
            in1=ev[:, :n], op0=mybir.AluOpType.mult, op1=mybir.AluOpType.add,
        )
    nc.scalar.activation(
        out=out_ap, in_=ev[:, :n],
        func=mybir.ActivationFunctionType.Prelu, alpha=alpha,
    )


def _kernel_body(tc, out_d, x_d, w3_d, wpw_d, vec_d, bl):
    nc = tc.nc
    ctx = ExitStack()
    with ctx:
        const = ctx.enter_context(tc.tile_pool(name="const", bufs=1))
        xpool = ctx.enter_context(tc.tile_pool(name="xpool", bufs=2))
        s1pool = ctx.enter_context(tc.tile_pool(name="s1pool", bufs=2))
        p1pool = ctx.enter_context(tc.tile_pool(name="p1pool", bufs=2))
        s2pool = ctx.enter_context(tc.tile_pool(name="s2pool", bufs=2))
        p2pool = ctx.enter_context(tc.tile_pool(name="p2pool", bufs=2))
        evpool = ctx.enter_context(tc.tile_pool(name="evpool", bufs=3))
        ps1 = ctx.enter_context(tc.tile_pool(name="ps1", bufs=2, space="PSUM"))
        ps2 = ctx.enter_context(tc.tile_pool(name="ps2", bufs=2, space="PSUM"))

        # ---- constants -------------------------------------------------
        v = {}
        for n in WVEC_NAMES:
            v[n] = const.tile([128, 1], F32, name=f"v_{n}")
            nc.sync.dma_start(out=v[n], in_=_bcast_ap(vec_d[n]))
        b31 = const.tile([128, 1], F32, name="b31")  # b13 + b21
        nc.vector.tensor_tensor(
            out=b31, in0=v["b13"], in1=v["b21"], op=mybir.AluOpType.add
        )
        b32 = const.tile([128, 1], F32, name="b32")  # b13 + b22
        nc.vector.tensor_tensor(
            out=b32, in0=v["b13"], in1=v["b22"], op=mybir.AluOpType.add
        )

        w1, as1, id1 = _prep_conv_consts(nc, const, w3_d, "c1")
        w2, as2, id2 = _prep_conv_consts(nc, const, wpw_d, "c2")

        # ---- main loop -------------------------------------------------
        X_ROWS = R + 4     # x / s1 strip rows   [h0-2, h0+R+2)
        P_ROWS = R + 2     # p1 / s2 strip rows  [h0-1, h0+R+1)
        X_LEN = X_ROWS * WP
        P_LEN = P_ROWS * WP

        for pair in range(bl // 2):
            imgs = (2 * pair, 2 * pair + 1)
            for s in range(NSTRIPS):
                h0 = s * R
                xlo, xhi = max(h0 - 2, 0), min(h0 + R + 2, H)
                c1lo, c1hi = max(h0 - 1, 0), min(h0 + R + 1, H)

                def xloc(g):   # global row -> local row in x/s1 strip
                    return g - (h0 - 2)

                def ploc(g):   # global row -> local row in p1/s2 strip
                    return g - (h0 - 1)

                # -- load x ---------------------------------------------
                x_t = xpool.tile([128, X_LEN + 4], F32, tag="x")
                x_r = x_t[:, 2 : 2 + X_LEN].rearrange(
                    "p (r c) -> p r c", c=WP
                )
                for j in range(2):
                    nc.sync.dma_start(
                        out=x_r[
                            64 * j : 64 * j + 64,
                            xloc(xlo) : xloc(xhi),
                            1 : 1 + W,
                        ],
                        in_=x_d[imgs[j], :, xlo:xhi, :],
                    )
                nc.gpsimd.memset(x_r[:, :, 0:1], 0.0)
                nc.gpsimd.memset(x_r[:, :, WP - 1 : WP], 0.0)

                # -- s1 = sign(x + b11), zero padding -------------------
                s1_t = s1pool.tile([128, X_LEN + 4], BF16, tag="s1")
                s1_r = s1_t[:, 2 : 2 + X_LEN].rearrange(
                    "p (r c) -> p r c", c=WP
                )
                nc.scalar.activation(
                    out=s1_t[:, 2 + xloc(xlo) * WP : 2 + xloc(xhi) * WP],
                    in_=x_t[:, 2 + xloc(xlo) * WP : 2 + xloc(xhi) * WP],
                    func=mybir.ActivationFunctionType.Sign,
                    bias=v["b11"],
                )
                nc.gpsimd.memset(s1_r[:, :, 0:1], 0.0)
                nc.gpsimd.memset(s1_r[:, :, WP - 1 : WP], 0.0)
                nc.gpsimd.memset(s1_t[:, 0:2], 0.0)
                nc.gpsimd.memset(s1_t[:, 2 + X_LEN :], 0.0)
                if xloc(xlo) > 0:  # top image edge
                    nc.gpsimd.memset(s1_t[:, 2 : 2 + xloc(xlo) * WP], 0.0)
                if xloc(xhi) < X_ROWS:  # bottom image edge
                    nc.gpsimd.memset(
                        s1_t[:, 2 + xloc(xhi) * WP : 2 + X_LEN], 0.0
                    )

                # -- conv1 + fused residual/scale/bias/prelu ------------
                p1_t = p1pool.tile([128, P_LEN + 4], F32, tag="p1")
                for r0, nr in _row_chunks(c1lo, c1hi):
                    _conv_bankset(
                        nc,
                        ps1.tile([128, 1024], F32, tag="ps1", name="pt1"),
                        w1, id1, s1_t,
                        soff=lambda t, _r=r0: 2
                        + (xloc(_r) + t // 3 - 1) * WP
                        + (t % 3 - 1),
                        res_t=x_t,
                        roff=2 + xloc(r0) * WP,
                        n=nr * WP,
                        asc=as1,
                        bias_mid=v["b12"],
                        alpha=v["a1"],
                        out_ap=p1_t[
                            :, 2 + ploc(r0) * WP : 2 + (ploc(r0) + nr) * WP
                        ],
                        ev=evpool.tile([128, 456], F32, tag="ev", name="ev"),
                    )

                # -- s2 = sign(p1 + b13 + b21), zero padding ------------
                s2_t = s2pool.tile([128, P_LEN + 4], BF16, tag="s2")
                s2_r = s2_t[:, 2 : 2 + P_LEN].rearrange(
                    "p (r c) -> p r c", c=WP
                )
                nc.scalar.activation(
                    out=s2_t[:, 2 + ploc(c1lo) * WP : 2 + ploc(c1hi) * WP],
                    in_=p1_t[:, 2 + ploc(c1lo) * WP : 2 + ploc(c1hi) * WP],
                    func=mybir.ActivationFunctionType.Sign,
                    bias=b31,
                )
                nc.gpsimd.memset(s2_r[:, :, 0:1], 0.0)
                nc.gpsimd.memset(s2_r[:, :, WP - 1 : WP], 0.0)
                nc.gpsimd.memset(s2_t[:, 0:2], 0.0)
                nc.gpsimd.memset(s2_t[:, 2 + P_LEN :], 0.0)
                if ploc(c1lo) > 0:
                    nc.gpsimd.memset(s2_t[:, 2 : 2 + ploc(c1lo) * WP], 0.0)
                if ploc(c1hi) < P_ROWS:
                    nc.gpsimd.memset(
                        s2_t[:, 2 + ploc(c1hi) * WP : 2 + P_LEN], 0.0
                    )

                # -- conv2 + fused chain --------------------------------
                p2_t = p2pool.tile([128, R * WP], F32, tag="p2")
                for r0, nr in _row_chunks(h0, h0 + R):
                    _conv_bankset(
                        nc,
                        ps2.tile([128, 1024], F32, tag="ps2", name="pt2"),
                        w2, id2, s2_t,
                        soff=lambda t, _r=r0: 2
                        + (ploc(_r) + t // 3 - 1) * WP
                        + (t % 3 - 1),
                        res_t=p1_t,
                        roff=2 + ploc(r0) * WP,
                        n=nr * WP,
                        asc=as2,
                        bias_mid=b32,
                        alpha=v["a2"],
                        out_ap=p2_t[
                            :, (r0 - h0) * WP : (r0 - h0 + nr) * WP
                        ],
                        ev=evpool.tile([128, 456], F32, tag="ev", name="ev"),
                    )

                # -- out2 = p2 + b23, store -----------------------------
                nc.vector.tensor_scalar_add(p2_t, p2_t, v["b23"])
                p2_r = p2_t.rearrange("p (r c) -> p r c", c=WP)
                for j in range(2):
                    nc.scalar.dma_start(
                        out=out_d[imgs[j], :, h0 : h0 + R, :],
                        in_=p2_r[64 * j : 64 * j + 64, :, 1 : 1 + W],
                    )


_NC_CACHE = {}


def _get_program(bl=BL):
    if bl not in _NC_CACHE:
        _NC_CACHE[bl] = build_program(bl)
    return _NC_CACHE[bl]


def make_in_maps(inputs):
    x = np.ascontiguousarray(np.asarray(inputs["x"], dtype=np.float32))
    shared = {
        "w3": np.ascontiguousarray(np.asarray(inputs["w3"], np.float32)),
        "w_pw": np.ascontiguousarray(np.asarray(inputs["w_pw"], np.float32)),
    }
    for n in WVEC_NAMES:
        shared[n] = np.ascontiguousarray(np.asarray(inputs[n], np.float32))
    return [{"x": x[i * BL : (i + 1) * BL], **shared} for i in range(NCORES)]


def run(inputs, trace=False, **kwargs):
    nc = _get_program(BL)
    res = run_bass_kernel_spmd(
        nc, make_in_maps(inputs), core_ids=list(range(NCORES)), trace=trace,
        **kwargs,
    )
    out = np.concatenate([r["out"] for r in res.results], axis=0)
    return out, res


def kernel(**inputs):
    return run(inputs)[0]


def bench(inputs, iters=20):
    """Steady-state wall-clock benchmark: sharded jit without donation,
    device-resident inputs, async dispatch of `iters` executions."""
    import time
    import jax
    from jax.sharding import Mesh, PartitionSpec, NamedSharding
    from jax.experimental.shard_map import shard_map
    from concourse import bass2jax as b2j

    b2j.install_neuronx_cc_hook()
    nc = _get_program(BL)
    in_maps = make_in_maps(inputs)

    in_names, out_names, out_avals = [], [], []
    for alloc in nc.m.functions[0].allocations:
        if not isinstance(mybir.MemoryLocationSet, type) or not isinstance(
            alloc, mybir.MemoryLocationSet
        ):
            continue
        name = alloc.memorylocations[0].name
        if alloc.kind == "ExternalInput":
            if nc.partition_id_tensor and name == nc.partition_id_tensor.name:
                continue
            in_names.append(name)
        elif alloc.kind == "ExternalOutput":
            out_names.append(name)
            out_avals.append(
                jax.core.ShapedArray(
                    tuple(alloc.tensor_shape), mybir.dt.np(alloc.dtype)
                )
            )
    n_params = len(in_names)
    all_names = in_names + out_names
    if nc.partition_id_tensor:
        all_names = all_names + [nc.partition_id_tensor.name]

    def _body(*args):
        operands = list(args)
        if nc.partition_id_tensor:
            operands.append(b2j.partition_id_tensor())
        outs = b2j._bass_exec_p.bind(
            *operands,
            out_avals=tuple(out_avals),
            in_names=tuple(all_names),
            out_names=tuple(out_names),
            lowering_input_output_aliases=(),
            sim_require_finite=True,
            sim_require_nnan=True,
            nc=nc,
        )
        return tuple(outs)

    devices = jax.devices()[:NCORES]
    mesh = Mesh(np.asarray(devices), ("core",))
    nin = n_params + len(out_names)
    f = jax.jit(
        shard_map(
            _body,
            mesh=mesh,
            in_specs=(PartitionSpec("core"),) * nin,
            out_specs=(PartitionSpec("core"),) * len(out_names),
            check_rep=False,
        ),
        keep_unused=True,
    )
    sh = NamedSharding(mesh, PartitionSpec("core"))
    concat_in = [
        jax.device_put(np.concatenate([m[n] for m in in_maps], axis=0), sh)
        for n in in_names
    ]
    zeros = [
        jax.device_put(
            np.zeros((NCORES * a.shape[0], *a.shape[1:]), a.dtype), sh
        )
        for a in out_avals
    ]

    r = f(*concat_in, *zeros)  # warm-up / compile
    jax.block_until_ready(r)

    res = {}
    for ntest in (1, 10, 30):
        ts = []
        for _ in range(3):
            t0 = time.perf_counter()
            rs = [f(*concat_in, *zeros) for _ in range(ntest)]
            jax.block_until_ready(rs)
            ts.append((time.perf_counter() - t0) / ntest)
        res[ntest] = min(ts)
    res["single_s"] = res[1]
    # slope between 10 and 30 removes the one-time dispatch ramp
    res["per_iter_s"] = (res[30] * 30 - res[10] * 10) / 20
    return res


if __name__ == "__main__":
    rng = np.random.default_rng(0)
    ins = {"x": rng.standard_normal((B, C, H, W)).astype(np.float32)}
    for n in ["w3", "w_pw"]:
        ins[n] = ((rng.random((C, C, 3, 3)) - 0.5) * 0.002).astype(np.float32)
    for n in WVEC_NAMES:
        ins[n] = (rng.standard_normal(C) * 0.01).astype(np.float32)
    out = kernel(**ins)
    print(out.shape, out.dtype)


# revision 22
# speedup vs baseline: 2.7625x; 2.7625x over previous
"""Trainium2 Bass kernel for a ReActNet-style binary BasicBlock.

Reference math per block (twice, with different weights):
    s   = sign(x + b_in)                      # +-1
    c   = conv3x3(s, mean|w| * sign(w))       # binarized conv, pad=1
    y   = x + ALPHA * c                       # residual
    y   = prelu(y + b_mid, a) + b_out

Key facts exploited:
  * matmul inputs are exactly +-1 -> bf16 matmuls are EXACT (integer sums
    accumulated in fp32 PSUM).
  * per-output-channel weight scale factors out:  conv(s, scale*sign(w)) =
    scale .* conv(s, sign(w)).
  * residual rides through PSUM via an fp32 matmul with diag(1/(ALPHA*scale)):
        T = binconv(s) + x / as           (as = ALPHA*scale, per channel)
    then prelu(x + as*binconv + b, a) = Prelu-activation(T) with
    per-partition scale=as, bias=b, alpha=a  -- a single ScalarE op reading
    PSUM directly.  (prelu positive homogeneity: as > 0.)

Layout: NCHW, channels (64) on partitions; 2 images stacked per 128
partitions (img even -> partitions 0-63, img odd -> 64-127).  Conv matmuls
run as two concurrent 64x64 PE tiles on the array diagonal.  Spatial strips
of R=16 output rows, width padded to 114 with zero columns held in the
sign tiles; conv taps are constant flat-offset shifts.

Sharding: pure data parallel, batch 32 -> 4 images x 8 cores, weights
replicated, no collectives.
"""

import numpy as np
from contextlib import ExitStack

import concourse.bass as bass
import concourse.tile as tile
from concourse import mybir
from concourse import bacc
from concourse.bass_utils import run_bass_kernel_spmd
from concourse.masks import make_identity

B, C, H, W = 32, 64, 112, 112
ALPHA = 0.25
NCORES = 8
BL = B // NCORES          # images per core
WP = W + 2                # padded width
R = 16                    # output rows per strip
NSTRIPS = H // R

F32 = mybir.dt.float32
BF16 = mybir.dt.bfloat16

WVEC_NAMES = ["b11", "b12", "b13", "b21", "b22", "b23", "a1", "a2"]

CONV_SCHEME = "orig"  # "quad" | "diag2" | "orig"


def _bcast_ap(dram_ap, reps=2):
    """Source AP replicating a DRAM tensor across partition groups."""
    return bass.AP(
        tensor=dram_ap.tensor,
        offset=dram_ap.offset,
        ap=[[0, reps]] + [list(d) for d in dram_ap.ap],
    )


def _row_chunks(lo, hi, step=4):
    r = lo
    while r < hi:
        yield r, min(step, hi - r)
        r += step


def build_program(bl=BL):
    """Build the Bass program for one core processing `bl` images."""
    nc = bacc.Bacc("TRN2", target_bir_lowering=False, debug=False)

    x_d = nc.dram_tensor("x", [bl, C, H, W], F32, kind="ExternalInput").ap()
    w3_d = nc.dram_tensor("w3", [C, C, 3, 3], F32, kind="ExternalInput").ap()
    wpw_d = nc.dram_tensor("w_pw", [C, C, 3, 3], F32, kind="ExternalInput").ap()
    vec_d = {
        n: nc.dram_tensor(n, [C], F32, kind="ExternalInput").ap()
        for n in WVEC_NAMES
    }
    out_d = nc.dram_tensor("out", [bl, C, H, W], F32, kind="ExternalOutput").ap()

    with tile.TileContext(nc) as tc:
        _kernel_body(tc, out_d, x_d, w3_d, wpw_d, vec_d, bl)

    nc.compile()
    return nc


def _prep_conv_consts(nc, const, wdram, name):
    """Per-conv constants: binarized-transposed weights, as=ALPHA*mean|w|,
    diag(1/as) for the residual matmul.  Everything replicated on both
    partition halves."""
    # natural layout [co, ci*3*3] duplicated -> per-channel scale
    wn = const.tile([128, C * 9], F32, name=f"wn_{name}")
    nc.sync.dma_start(out=wn, in_=_bcast_ap(wdram.rearrange("a b c d -> a (b c d)")))
    wabs = const.tile([128, C * 9], F32, name=f"wabs_{name}")
    asum = const.tile([128, 1], F32, name=f"asum_{name}")
    nc.scalar.activation(
        out=wabs, in_=wn, func=mybir.ActivationFunctionType.Abs, accum_out=asum
    )
    asc = const.tile([128, 1], F32, name=f"asc_{name}")  # ALPHA * mean|w|
    nc.vector.tensor_scalar_mul(asc, asum, ALPHA / (C * 9))
    inv_asc = const.tile([128, 1], F32, name=f"iasc_{name}")
    nc.vector.reciprocal(inv_asc, asc)

    # gathered+transposed weights [ci(+dup), co, tap], then binarize to bf16
    wg = const.tile([128, C, 9], F32, name=f"wg_{name}")
    src = bass.AP(
        tensor=wdram.tensor,
        offset=wdram.offset,
        ap=[[9, C], [C * 9, C], [1, 9]],
    )
    for rep in range(2):
        nc.sync.dma_start(out=wg[64 * rep : 64 * rep + 64, :, :], in_=src)
    wsign = const.tile([128, C, 9], BF16, name=f"ws_{name}")
    nc.scalar.activation(out=wsign, in_=wg, func=mybir.ActivationFunctionType.Sign)

    # residual injector: diag(1/as) fp32, per partition half
    ident = const.tile([128, C], F32, name=f"id_{name}")
    make_identity(nc, ident[0:64, :])
    make_identity(nc, ident[64:128, :])
    nc.vector.tensor_scalar_mul(ident, ident, inv_asc)
    return wsign, asc, ident


def _conv_bankset(nc, pt, w, ident, s_t, soff, res_t, roff, n, asc, bias_mid,
                  alpha, out_ap, ev):
    """One 2-bank PSUM set covering both images of the pair.

    Four concurrent 64x64 PE quadrants:
      (row 0, col 0): imgA subset1 -> bank0[0:64]
      (row 64, col 64): imgB subset1 -> bank0[64:128]
      (row 0, col 64): imgA subset2 -> bank1[64:128]   (crossed)
      (row 64, col 0): imgB subset2 -> bank1[0:64]     (crossed)
    subset1 = {residual-injector matmul, taps 0-3}; subset2 = {taps 4-8}.
    Eviction: u = as*bank0 + bias (ACT, full width); u += as*bank1 via two
    half-width cross-partition STTs (DVE); out = Prelu(u) (ACT).
    """
    lo, hi = slice(0, 64), slice(64, 128)
    if CONV_SCHEME == "orig":
        # single-bank psum, serial taps per half, Prelu direct from PSUM
        for rsl in (lo, hi):
            nc.tensor.matmul(
                pt[rsl, :n], ident[rsl, :], res_t[rsl, roff : roff + n],
                start=True, stop=False, skip_group_check=True,
            )
            for t in range(9):
                nc.tensor.matmul(
                    pt[rsl, :n], w[rsl, :, t], s_t[rsl, soff(t) : soff(t) + n],
                    start=False, stop=(t == 8), skip_group_check=True,
                )
        nc.scalar.activation(
            out=out_ap, in_=pt[:, :n],
            func=mybir.ActivationFunctionType.Prelu,
            bias=bias_mid, scale=asc, alpha=alpha,
        )
        return
    if CONV_SCHEME == "quad":
        quads = [(lo, lo, 0), (hi, hi, 0), (lo, hi, 1), (hi, lo, 1)]
    else:  # diag2: only diagonal quadrants, banks aligned
        quads = [(lo, lo, 0), (hi, hi, 0), (lo, lo, 1), (hi, hi, 1)]
    slots = (
        [("id", None), ("tap", 0), ("tap", 1), ("tap", 2), ("tap", 3)],
        [("tap", 4), ("tap", 5), ("tap", 6), ("tap", 7), ("tap", 8)],
    )
    for s in range(5):
        for rsl, osl, sub in quads:
            kind, t = slots[sub][s]
            boff = 0 if sub == 0 else 512
            out = pt[osl, boff : boff + n]
            if kind == "id":
                nc.tensor.matmul(
                    out, ident[rsl, :], res_t[rsl, roff : roff + n],
                    start=True, stop=False, skip_group_check=True,
                )
            else:
                nc.tensor.matmul(
                    out, w[rsl, :, t], s_t[rsl, soff(t) : soff(t) + n],
                    start=(s == 0), stop=(s == 4), skip_group_check=True,
                )
    nc.scalar.activation(
        out=ev[:, :n], in_=pt[:, 0:n],
        func=mybir.ActivationFunctionType.Identity, bias=bias_mid, scale=asc,
    )
    if CONV_SCHEME == "quad":
        nc.vector.scalar_tensor_tensor(
            out=ev[0:64, :n], in0=pt[64:128, 512 : 512 + n], scalar=asc[0:64],
            in1=ev[0:64, :n], op0=mybir.AluOpType.mult, op1=mybir.AluOpType.add,
        )
        nc.vector.scalar_tensor_tensor(
            out=ev[64:128, :n], in0=pt[0:64, 512 : 512 + n],
            scalar=asc[64:128], in1=ev[64:128, :n],
            op0=mybir.AluOpType.mult, op1=mybir.AluOpType.add,
        )
    else:
        nc.vector.scalar_tensor_tensor(
            out=ev[:, :n], in0=pt[:, 512 : 512 + n], scalar=asc,
            in1=ev[:, :n], op0=mybir.AluOpType.mult, op1=mybir.AluOpType.add,
        )
    nc.scalar.activation(
        out=out_ap, in_=ev[:, :n],
        func=mybir.ActivationFunctionType.Prelu, alpha=alpha,
    )


def _kernel_body(tc, out_d, x_d, w3_d, wpw_d, vec_d, bl):
    nc = tc.nc
    ctx = ExitStack()
    with ctx:
        const = ctx.enter_context(tc.tile_pool(name="const", bufs=1))
        xpool = ctx.enter_context(tc.tile_pool(name="xpool", bufs=2))
        s1pool = ctx.enter_context(tc.tile_pool(name="s1pool", bufs=2))
        p1pool = ctx.enter_context(tc.tile_pool(name="p1pool", bufs=2))
        s2pool = ctx.enter_context(tc.tile_pool(name="s2pool", bufs=2))
        p2pool = ctx.enter_context(tc.tile_pool(name="p2pool", bufs=2))
        evpool = ctx.enter_context(tc.tile_pool(name="evpool", bufs=3))
        ps1 = ctx.enter_context(tc.tile_pool(name="ps1", bufs=2, space="PSUM"))
        ps2 = ctx.enter_context(tc.tile_pool(name="ps2", bufs=2, space="PSUM"))

        # ---- constants -------------------------------------------------
        v = {}
        for n in WVEC_NAMES:
            v[n] = const.tile([128, 1], F32, name=f"v_{n}")
            nc.sync.dma_start(out=v[n], in_=_bcast_ap(vec_d[n]))
        b31 = const.tile([128, 1], F32, name="b31")  # b13 + b21
        nc.vector.tensor_tensor(
            out=b31, in0=v["b13"], in1=v["b21"], op=mybir.AluOpType.add
        )
        b32 = const.tile([128, 1], F32, name="b32")  # b13 + b22
        nc.vector.tensor_tensor(
            out=b32, in0=v["b13"], in1=v["b22"], op=mybir.AluOpType.add
        )

        w1, as1, id1 = _prep_conv_consts(nc, const, w3_d, "c1")
        w2, as2, id2 = _prep_conv_consts(nc, const, wpw_d, "c2")

        # ---- main loop -------------------------------------------------
        X_ROWS = R + 4     # x / s1 strip rows   [h0-2, h0+R+2)
        P_ROWS = R + 2     # p1 / s2 strip rows  [h0-1, h0+R+1)
        X_LEN = X_ROWS * WP
        P_LEN = P_ROWS * WP

        for pair in range(bl // 2):
            imgs = (2 * pair, 2 * pair + 1)
            for s in range(NSTRIPS):
                h0 = s * R
                xlo, xhi = max(h0 - 2, 0), min(h0 + R + 2, H)
                c1lo, c1hi = max(h0 - 1, 0), min(h0 + R + 1, H)

                def xloc(g):   # global row -> local row in x/s1 strip
                    return g - (h0 - 2)

                def ploc(g):   # global row -> local row in p1/s2 strip
                    return g - (h0 - 1)

                # -- load x ---------------------------------------------
                x_t = xpool.tile([128, X_LEN + 4], F32, tag="x")
                x_r = x_t[:, 2 : 2 + X_LEN].rearrange(
                    "p (r c) -> p r c", c=WP
                )
                for j in range(2):
                    nc.sync.dma_start(
                        out=x_r[
                            64 * j : 64 * j + 64,
                            xloc(xlo) : xloc(xhi),
                            1 : 1 + W,
                        ],
                        in_=x_d[imgs[j], :, xlo:xhi, :],
                    )
                nc.gpsimd.memset(x_r[:, :, 0:1], 0.0)
                nc.gpsimd.memset(x_r[:, :, WP - 1 : WP], 0.0)

                # -- s1 = sign(x + b11), zero padding -------------------
                s1_t = s1pool.tile([128, X_LEN + 4], BF16, tag="s1")
                s1_r = s1_t[:, 2 : 2 + X_LEN].rearrange(
                    "p (r c) -> p r c", c=WP
                )
                nc.scalar.activation(
                    out=s1_t[:, 2 + xloc(xlo) * WP : 2 + xloc(xhi) * WP],
                    in_=x_t[:, 2 + xloc(xlo) * WP : 2 + xloc(xhi) * WP],
                    func=mybir.ActivationFunctionType.Sign,
                    bias=v["b11"],
                )
                nc.gpsimd.memset(s1_r[:, :, 0:1], 0.0)
                nc.gpsimd.memset(s1_r[:, :, WP - 1 : WP], 0.0)
                nc.gpsimd.memset(s1_t[:, 0:2], 0.0)
                nc.gpsimd.memset(s1_t[:, 2 + X_LEN :], 0.0)
                if xloc(xlo) > 0:  # top image edge
                    nc.gpsimd.memset(s1_t[:, 2 : 2 + xloc(xlo) * WP], 0.0)
                if xloc(xhi) < X_ROWS:  # bottom image edge
                    nc.gpsimd.memset(
                        s1_t[:, 2 + xloc(xhi) * WP : 2 + X_LEN], 0.0
                    )

                # -- conv1 + fused residual/scale/bias/prelu ------------
                p1_t = p1pool.tile([128, P_LEN + 4], F32, tag="p1")
                for r0, nr in _row_chunks(c1lo, c1hi):
                    _conv_bankset(
                        nc,
                        ps1.tile([128, 1024], F32, tag="ps1", name="pt1"),
                        w1, id1, s1_t,
                        soff=lambda t, _r=r0: 2
                        + (xloc(_r) + t // 3 - 1) * WP
                        + (t % 3 - 1),
                        res_t=x_t,
                        roff=2 + xloc(r0) * WP,
                        n=nr * WP,
                        asc=as1,
                        bias_mid=v["b12"],
                        alpha=v["a1"],
                        out_ap=p1_t[
                            :, 2 + ploc(r0) * WP : 2 + (ploc(r0) + nr) * WP
                        ],
                        ev=evpool.tile([128, 456], F32, tag="ev", name="ev"),
                    )

                # -- s2 = sign(p1 + b13 + b21), zero padding ------------
                s2_t = s2pool.tile([128, P_LEN + 4], BF16, tag="s2")
                s2_r = s2_t[:, 2 : 2 + P_LEN].rearrange(
                    "p (r c) -> p r c", c=WP
                )
                nc.scalar.activation(
                    out=s2_t[:, 2 + ploc(c1lo) * WP : 2 + ploc(c1hi) * WP],
                    in_=p1_t[:, 2 + ploc(c1lo) * WP : 2 + ploc(c1hi) * WP],
                    func=mybir.ActivationFunctionType.Sign,
                    bias=b31,
                )
                nc.gpsimd.memset(s2_r[:, :, 0:1], 0.0)
                nc.gpsimd.memset(s2_r[:, :, WP - 1 : WP], 0.0)
                nc.gpsimd.memset(s2_t[:, 0:2], 0.0)
                nc.gpsimd.memset(s2_t[:, 2 + P_LEN :], 0.0)
                if ploc(c1lo) > 0:
                    nc.gpsimd.memset(s2_t[:, 2 : 2 + ploc(c1lo) * WP], 0.0)
                if ploc(c1hi) < P_ROWS:
                    nc.gpsimd.memset(
                        s2_t[:, 2 + ploc(c1hi) * WP : 2 + P_LEN], 0.0
                    )

                # -- conv2 + fused chain --------------------------------
                p2_t = p2pool.tile([128, R * WP], F32, tag="p2")
                for r0, nr in _row_chunks(h0, h0 + R):
                    _conv_bankset(
                        nc,
                        ps2.tile([128, 1024], F32, tag="ps2", name="pt2"),
                        w2, id2, s2_t,
                        soff=lambda t, _r=r0: 2
                        + (ploc(_r) + t // 3 - 1) * WP
                        + (t % 3 - 1),
                        res_t=p1_t,
                        roff=2 + ploc(r0) * WP,
                        n=nr * WP,
                        asc=as2,
                        bias_mid=b32,
                        alpha=v["a2"],
                        out_ap=p2_t[
                            :, (r0 - h0) * WP : (r0 - h0 + nr) * WP
                        ],
                        ev=evpool.tile([128, 456], F32, tag="ev", name="ev"),
                    )

                # -- out2 = p2 + b23, store -----------------------------
                nc.vector.tensor_scalar_add(p2_t, p2_t, v["b23"])
                p2_r = p2_t.rearrange("p (r c) -> p r c", c=WP)
                for j in range(2):
                    nc.scalar.dma_start(
                        out=out_d[imgs[j], :, h0 : h0 + R, :],
                        in_=p2_r[64 * j : 64 * j + 64, :, 1 : 1 + W],
                    )


_NC_CACHE = {}


def _get_program(bl=BL):
    if bl not in _NC_CACHE:
        _NC_CACHE[bl] = build_program(bl)
    return _NC_CACHE[bl]


def make_in_maps(inputs):
    x = np.ascontiguousarray(np.asarray(inputs["x"], dtype=np.float32))
    shared = {
        "w3": np.ascontiguousarray(np.asarray(inputs["w3"], np.float32)),
        "w_pw": np.ascontiguousarray(np.asarray(inputs["w_pw"], np.float32)),
    }
    for n in WVEC_NAMES:
        shared[n] = np.ascontiguousarray(np.asarray(inputs[n], np.float32))
    return [{"x": x[i * BL : (i + 1) * BL], **shared} for i in range(NCORES)]


def run(inputs, trace=False, **kwargs):
    nc = _get_program(BL)
    res = run_bass_kernel_spmd(
        nc, make_in_maps(inputs), core_ids=list(range(NCORES)), trace=trace,
        **kwargs,
    )
    out = np.concatenate([r["out"] for r in res.results], axis=0)
    return out, res


def kernel(**inputs):
    return run(inputs)[0]


def bench(inputs, iters=20):
    """Steady-state wall-clock benchmark: sharded jit without donation,
    device-resident inputs, async dispatch of `iters` executions."""
    import time
    import jax
    from jax.sharding import Mesh, PartitionSpec, NamedSharding
    from jax.experimental.shard_map import shard_map
    from concourse import bass2jax as b2j

    b2j.install_neuronx_cc_hook()
    nc = _get_program(BL)
    in_maps = make_in_maps(inputs)

    in_names, out_names, out_avals = [], [], []
    for alloc in nc.m.functions[0].allocations:
        if not isinstance(mybir.MemoryLocationSet, type) or not isinstance(
            alloc, mybir.MemoryLocationSet
        ):
            continue
        name = alloc.memorylocations[0].name
        if alloc.kind == "ExternalInput":
            if nc.partition_id_tensor and name == nc.partition_id_tensor.name:
                continue
            in_names.append(name)
        elif alloc.kind == "ExternalOutput":
            out_names.append(name)
            out_avals.append(
                jax.core.ShapedArray(
                    tuple(alloc.tensor_shape), mybir.dt.np(alloc.dtype)
                )
            )
    n_params = len(in_names)
    all_names = in_names + out_names
    if nc.partition_id_tensor:
        all_names = all_names + [nc.partition_id_tensor.name]

    def _body(*args):
        operands = list(args)
        if nc.partition_id_tensor:
            operands.append(b2j.partition_id_tensor())
        outs = b2j._bass_exec_p.bind(
            *operands,
            out_avals=tuple(out_avals),
            in_names=tuple(all_names),
            out_names=tuple(out_names),
            lowering_input_output_aliases=(),
            sim_require_finite=True,
            sim_require_nnan=True,
            nc=nc,
        )
        return tuple(outs)

    devices = jax.devices()[:NCORES]
    mesh = Mesh(np.asarray(devices), ("core",))
    nin = n_params + len(out_names)
    f = jax.jit(
        shard_map(
            _body,
            mesh=mesh,
            in_specs=(PartitionSpec("core"),) * nin,
            out_specs=(PartitionSpec("core"),) * len(out_names),
            check_rep=False,
        ),
        keep_unused=True,
    )
    sh = NamedSharding(mesh, PartitionSpec("core"))
    concat_in = [
        jax.device_put(np.concatenate([m[n] for m in in_maps], axis=0), sh)
        for n in in_names
    ]
    zeros = [
        jax.device_put(
            np.zeros((NCORES * a.shape[0], *a.shape[1:]), a.dtype), sh
        )
        for a in out_avals
    ]

    r = f(*concat_in, *zeros)  # warm-up / compile
    jax.block_until_ready(r)

    res = {}
    for ntest in (1, 10, 30):
        ts = []
        for _ in range(3):
            t0 = time.perf_counter()
            rs = [f(*concat_in, *zeros) for _ in range(ntest)]
            jax.block_until_ready(rs)
            ts.append((time.perf_counter() - t0) / ntest)
        res[ntest] = min(ts)
    res["single_s"] = res[1]
    # slope between 10 and 30 removes the one-time dispatch ramp
    res["per_iter_s"] = (res[30] * 30 - res[10] * 10) / 20
    return res


if __name__ == "__main__":
    rng = np.random.default_rng(0)
    ins = {"x": rng.standard_normal((B, C, H, W)).astype(np.float32)}
    for n in ["w3", "w_pw"]:
        ins[n] = ((rng.random((C, C, 3, 3)) - 0.5) * 0.002).astype(np.float32)
    for n in WVEC_NAMES:
        ins[n] = (rng.standard_normal(C) * 0.01).astype(np.float32)
    out = kernel(**ins)
    print(out.shape, out.dtype)


# revision 23
# speedup vs baseline: 2.9223x; 1.0579x over previous
"""Trainium2 Bass kernel for a ReActNet-style binary BasicBlock.

Reference math per block (twice, with different weights):
    s   = sign(x + b_in)                      # +-1
    c   = conv3x3(s, mean|w| * sign(w))       # binarized conv, pad=1
    y   = x + ALPHA * c                       # residual
    y   = prelu(y + b_mid, a) + b_out

Key facts exploited:
  * matmul inputs are exactly +-1 -> bf16 matmuls are EXACT (integer sums
    accumulated in fp32 PSUM).
  * per-output-channel weight scale factors out:  conv(s, scale*sign(w)) =
    scale .* conv(s, sign(w)).
  * residual rides through PSUM via an fp32 matmul with diag(1/(ALPHA*scale)):
        T = binconv(s) + x / as           (as = ALPHA*scale, per channel)
    then prelu(x + as*binconv + b, a) = Prelu-activation(T) with
    per-partition scale=as, bias=b, alpha=a  -- a single ScalarE op reading
    PSUM directly.  (prelu positive homogeneity: as > 0.)

Layout: NCHW, channels (64) on partitions; 2 images stacked per 128
partitions (img even -> partitions 0-63, img odd -> 64-127).  Conv matmuls
run as two concurrent 64x64 PE tiles on the array diagonal.  Spatial strips
of R=16 output rows, width padded to 114 with zero columns held in the
sign tiles; conv taps are constant flat-offset shifts.

Sharding: pure data parallel, batch 32 -> 4 images x 8 cores, weights
replicated, no collectives.
"""

import numpy as np
from contextlib import ExitStack

import concourse.bass as bass
import concourse.tile as tile
from concourse import mybir
from concourse import bacc
from concourse.bass_utils import run_bass_kernel_spmd
from concourse.masks import make_identity

B, C, H, W = 32, 64, 112, 112
ALPHA = 0.25
NCORES = 8
BL = B // NCORES          # images per core
WP = W + 2                # padded width
R = 16                    # output rows per strip
NSTRIPS = H // R

F32 = mybir.dt.float32
BF16 = mybir.dt.bfloat16

WVEC_NAMES = ["b11", "b12", "b13", "b21", "b22", "b23", "a1", "a2"]

CONV_SCHEME = "orig"  # "quad" | "diag2" | "orig"


def _bcast_ap(dram_ap, reps=2):
    """Source AP replicating a DRAM tensor across partition groups."""
    return bass.AP(
        tensor=dram_ap.tensor,
        offset=dram_ap.offset,
        ap=[[0, reps]] + [list(d) for d in dram_ap.ap],
    )


def _row_chunks(lo, hi, step=4):
    r = lo
    while r < hi:
        yield r, min(step, hi - r)
        r += step


def build_program(bl=BL):
    """Build the Bass program for one core processing `bl` images."""
    nc = bacc.Bacc("TRN2", target_bir_lowering=False, debug=False)

    x_d = nc.dram_tensor("x", [bl, C, H, W], F32, kind="ExternalInput").ap()
    w3_d = nc.dram_tensor("w3", [C, C, 3, 3], F32, kind="ExternalInput").ap()
    wpw_d = nc.dram_tensor("w_pw", [C, C, 3, 3], F32, kind="ExternalInput").ap()
    vec_d = {
        n: nc.dram_tensor(n, [C], F32, kind="ExternalInput").ap()
        for n in WVEC_NAMES
    }
    out_d = nc.dram_tensor("out", [bl, C, H, W], F32, kind="ExternalOutput").ap()

    with tile.TileContext(nc) as tc:
        _kernel_body(tc, out_d, x_d, w3_d, wpw_d, vec_d, bl)

    nc.compile()
    return nc


def _prep_conv_consts(nc, const, wdram, name):
    """Per-conv constants: binarized-transposed weights, as=ALPHA*mean|w|,
    diag(1/as) for the residual matmul.  Everything replicated on both
    partition halves."""
    # natural layout [co, ci*3*3] duplicated -> per-channel scale
    wn = const.tile([128, C * 9], F32, name=f"wn_{name}")
    nc.sync.dma_start(out=wn, in_=_bcast_ap(wdram.rearrange("a b c d -> a (b c d)")))
    wabs = const.tile([128, C * 9], F32, name=f"wabs_{name}")
    asum = const.tile([128, 1], F32, name=f"asum_{name}")
    nc.scalar.activation(
        out=wabs, in_=wn, func=mybir.ActivationFunctionType.Abs, accum_out=asum
    )
    asc = const.tile([128, 1], F32, name=f"asc_{name}")  # ALPHA * mean|w|
    nc.vector.tensor_scalar_mul(asc, asum, ALPHA / (C * 9))
    inv_asc = const.tile([128, 1], F32, name=f"iasc_{name}")
    nc.vector.reciprocal(inv_asc, asc)

    # gathered+transposed weights [ci(+dup), co, tap], then binarize to bf16
    wg = const.tile([128, C, 9], F32, name=f"wg_{name}")
    src = bass.AP(
        tensor=wdram.tensor,
        offset=wdram.offset,
        ap=[[9, C], [C * 9, C], [1, 9]],
    )
    for rep in range(2):
        nc.sync.dma_start(out=wg[64 * rep : 64 * rep + 64, :, :], in_=src)
    wsign = const.tile([128, C, 9], BF16, name=f"ws_{name}")
    nc.scalar.activation(out=wsign, in_=wg, func=mybir.ActivationFunctionType.Sign)

    # residual injector: diag(1/as) fp32, per partition half
    ident = const.tile([128, C], F32, name=f"id_{name}")
    make_identity(nc, ident[0:64, :])
    make_identity(nc, ident[64:128, :])
    nc.vector.tensor_scalar_mul(ident, ident, inv_asc)
    return wsign, asc, ident


def _conv_bankset(nc, pt, w, ident, s_t, soff, res_t, roff, n, asc, bias_mid,
                  alpha, out_ap, ev):
    """One 2-bank PSUM set covering both images of the pair.

    Four concurrent 64x64 PE quadrants:
      (row 0, col 0): imgA subset1 -> bank0[0:64]
      (row 64, col 64): imgB subset1 -> bank0[64:128]
      (row 0, col 64): imgA subset2 -> bank1[64:128]   (crossed)
      (row 64, col 0): imgB subset2 -> bank1[0:64]     (crossed)
    subset1 = {residual-injector matmul, taps 0-3}; subset2 = {taps 4-8}.
    Eviction: u = as*bank0 + bias (ACT, full width); u += as*bank1 via two
    half-width cross-partition STTs (DVE); out = Prelu(u) (ACT).
    """
    lo, hi = slice(0, 64), slice(64, 128)
    if CONV_SCHEME == "orig":
        # single-bank psum, serial taps per half, Prelu direct from PSUM
        for rsl in (lo, hi):
            nc.tensor.matmul(
                pt[rsl, :n], ident[rsl, :], res_t[rsl, roff : roff + n],
                start=True, stop=False, skip_group_check=True,
            )
            for t in range(9):
                nc.tensor.matmul(
                    pt[rsl, :n], w[rsl, :, t], s_t[rsl, soff(t) : soff(t) + n],
                    start=False, stop=(t == 8), skip_group_check=True,
                )
        nc.scalar.activation(
            out=out_ap, in_=pt[:, :n],
            func=mybir.ActivationFunctionType.Prelu,
            bias=bias_mid, scale=asc, alpha=alpha,
        )
        return
    if CONV_SCHEME == "quad":
        quads = [(lo, lo, 0), (hi, hi, 0), (lo, hi, 1), (hi, lo, 1)]
    else:  # diag2: only diagonal quadrants, banks aligned
        quads = [(lo, lo, 0), (hi, hi, 0), (lo, lo, 1), (hi, hi, 1)]
    slots = (
        [("id", None), ("tap", 0), ("tap", 1), ("tap", 2), ("tap", 3)],
        [("tap", 4), ("tap", 5), ("tap", 6), ("tap", 7), ("tap", 8)],
    )
    for s in range(5):
        for rsl, osl, sub in quads:
            kind, t = slots[sub][s]
            boff = 0 if sub == 0 else 512
            out = pt[osl, boff : boff + n]
            if kind == "id":
                nc.tensor.matmul(
                    out, ident[rsl, :], res_t[rsl, roff : roff + n],
                    start=True, stop=False, skip_group_check=True,
                )
            else:
                nc.tensor.matmul(
                    out, w[rsl, :, t], s_t[rsl, soff(t) : soff(t) + n],
                    start=(s == 0), stop=(s == 4), skip_group_check=True,
                )
    nc.scalar.activation(
        out=ev[:, :n], in_=pt[:, 0:n],
        func=mybir.ActivationFunctionType.Identity, bias=bias_mid, scale=asc,
    )
    if CONV_SCHEME == "quad":
        nc.vector.scalar_tensor_tensor(
            out=ev[0:64, :n], in0=pt[64:128, 512 : 512 + n], scalar=asc[0:64],
            in1=ev[0:64, :n], op0=mybir.AluOpType.mult, op1=mybir.AluOpType.add,
        )
        nc.vector.scalar_tensor_tensor(
            out=ev[64:128, :n], in0=pt[0:64, 512 : 512 + n],
            scalar=asc[64:128], in1=ev[64:128, :n],
            op0=mybir.AluOpType.mult, op1=mybir.AluOpType.add,
        )
    else:
        nc.vector.scalar_tensor_tensor(
            out=ev[:, :n], in0=pt[:, 512 : 512 + n], scalar=asc,
            in1=ev[:, :n], op0=mybir.AluOpType.mult, op1=mybir.AluOpType.add,
        )
    nc.scalar.activation(
        out=out_ap, in_=ev[:, :n],
        func=mybir.ActivationFunctionType.Prelu, alpha=alpha,
    )


def _kernel_body(tc, out_d, x_d, w3_d, wpw_d, vec_d, bl):
    nc = tc.nc
    ctx = ExitStack()
    with ctx:
        const = ctx.enter_context(tc.tile_pool(name="const", bufs=1))
        xpool = ctx.enter_context(tc.tile_pool(name="xpool", bufs=2))
        s1pool = ctx.enter_context(tc.tile_pool(name="s1pool", bufs=2))
        p1pool = ctx.enter_context(tc.tile_pool(name="p1pool", bufs=2))
        s2pool = ctx.enter_context(tc.tile_pool(name="s2pool", bufs=2))
        p2pool = ctx.enter_context(tc.tile_pool(name="p2pool", bufs=2))
        evpool = ctx.enter_context(tc.tile_pool(name="evpool", bufs=3))
        ps1 = ctx.enter_context(tc.tile_pool(name="ps1", bufs=2, space="PSUM"))
        ps2 = ctx.enter_context(tc.tile_pool(name="ps2", bufs=2, space="PSUM"))

        # ---- constants -------------------------------------------------
        v = {}
        for n in WVEC_NAMES:
            v[n] = const.tile([128, 1], F32, name=f"v_{n}")
            nc.sync.dma_start(out=v[n], in_=_bcast_ap(vec_d[n]))
        b31 = const.tile([128, 1], F32, name="b31")  # b13 + b21
        nc.vector.tensor_tensor(
            out=b31, in0=v["b13"], in1=v["b21"], op=mybir.AluOpType.add
        )
        b32 = const.tile([128, 1], F32, name="b32")  # b13 + b22
        nc.vector.tensor_tensor(
            out=b32, in0=v["b13"], in1=v["b22"], op=mybir.AluOpType.add
        )

        w1, as1, id1 = _prep_conv_consts(nc, const, w3_d, "c1")
        w2, as2, id2 = _prep_conv_consts(nc, const, wpw_d, "c2")

        # ---- main loop -------------------------------------------------
        X_ROWS = R + 4     # x / s1 strip rows   [h0-2, h0+R+2)
        P_ROWS = R + 2     # p1 / s2 strip rows  [h0-1, h0+R+1)
        X_LEN = X_ROWS * WP
        P_LEN = P_ROWS * WP

        for pair in range(bl // 2):
            imgs = (2 * pair, 2 * pair + 1)
            for s in range(NSTRIPS):
                h0 = s * R
                xlo, xhi = max(h0 - 2, 0), min(h0 + R + 2, H)
                c1lo, c1hi = max(h0 - 1, 0), min(h0 + R + 1, H)

                def xloc(g):   # global row -> local row in x/s1 strip
                    return g - (h0 - 2)

                def ploc(g):   # global row -> local row in p1/s2 strip
                    return g - (h0 - 1)

                # -- load x ---------------------------------------------
                x_t = xpool.tile([128, X_LEN + 4], F32, tag="x")
                x_r = x_t[:, 2 : 2 + X_LEN].rearrange(
                    "p (r c) -> p r c", c=WP
                )
                for j in range(2):
                    nc.sync.dma_start(
                        out=x_r[
                            64 * j : 64 * j + 64,
                            xloc(xlo) : xloc(xhi),
                            1 : 1 + W,
                        ],
                        in_=x_d[imgs[j], :, xlo:xhi, :],
                    )
                nc.gpsimd.memset(x_r[:, :, 0:1], 0.0)
                nc.gpsimd.memset(x_r[:, :, WP - 1 : WP], 0.0)

                # -- s1 = sign(x + b11), zero padding -------------------
                s1_t = s1pool.tile([128, X_LEN + 4], BF16, tag="s1")
                s1_r = s1_t[:, 2 : 2 + X_LEN].rearrange(
                    "p (r c) -> p r c", c=WP
                )
                nc.scalar.activation(
                    out=s1_t[:, 2 + xloc(xlo) * WP : 2 + xloc(xhi) * WP],
                    in_=x_t[:, 2 + xloc(xlo) * WP : 2 + xloc(xhi) * WP],
                    func=mybir.ActivationFunctionType.Sign,
                    bias=v["b11"],
                )
                nc.gpsimd.memset(s1_r[:, :, 0:1], 0.0)
                nc.gpsimd.memset(s1_r[:, :, WP - 1 : WP], 0.0)
                nc.gpsimd.memset(s1_t[:, 0:2], 0.0)
                nc.gpsimd.memset(s1_t[:, 2 + X_LEN :], 0.0)
                if xloc(xlo) > 0:  # top image edge
                    nc.gpsimd.memset(s1_t[:, 2 : 2 + xloc(xlo) * WP], 0.0)
                if xloc(xhi) < X_ROWS:  # bottom image edge
                    nc.gpsimd.memset(
                        s1_t[:, 2 + xloc(xhi) * WP : 2 + X_LEN], 0.0
                    )

                # -- conv1 + fused residual/scale/bias/prelu ------------
                p1_t = p1pool.tile([128, P_LEN + 4], F32, tag="p1")
                for r0, nr in _row_chunks(c1lo, c1hi):
                    _conv_bankset(
                        nc,
                        ps1.tile([128, 1024], F32, tag="ps1", name="pt1"),
                        w1, id1, s1_t,
                        soff=lambda t, _r=r0: 2
                        + (xloc(_r) + t // 3 - 1) * WP
                        + (t % 3 - 1),
                        res_t=x_t,
                        roff=2 + xloc(r0) * WP,
                        n=nr * WP,
                        asc=as1,
                        bias_mid=v["b12"],
                        alpha=v["a1"],
                        out_ap=p1_t[
                            :, 2 + ploc(r0) * WP : 2 + (ploc(r0) + nr) * WP
                        ],
                        ev=evpool.tile([128, 456], F32, tag="ev", name="ev"),
                    )

                # -- s2 = sign(p1 + b13 + b21), zero padding ------------
                s2_t = s2pool.tile([128, P_LEN + 4], BF16, tag="s2")
                s2_r = s2_t[:, 2 : 2 + P_LEN].rearrange(
                    "p (r c) -> p r c", c=WP
                )
                nc.scalar.activation(
                    out=s2_t[:, 2 + ploc(c1lo) * WP : 2 + ploc(c1hi) * WP],
                    in_=p1_t[:, 2 + ploc(c1lo) * WP : 2 + ploc(c1hi) * WP],
                    func=mybir.ActivationFunctionType.Sign,
                    bias=b31,
                )
                nc.gpsimd.memset(s2_r[:, :, 0:1], 0.0)
                nc.gpsimd.memset(s2_r[:, :, WP - 1 : WP], 0.0)
                nc.gpsimd.memset(s2_t[:, 0:2], 0.0)
                nc.gpsimd.memset(s2_t[:, 2 + P_LEN :], 0.0)
                if ploc(c1lo) > 0:
                    nc.gpsimd.memset(s2_t[:, 2 : 2 + ploc(c1lo) * WP], 0.0)
                if ploc(c1hi) < P_ROWS:
                    nc.gpsimd.memset(
                        s2_t[:, 2 + ploc(c1hi) * WP : 2 + P_LEN], 0.0
                    )

                # -- conv2 + fused chain --------------------------------
                p2_t = p2pool.tile([128, R * WP], F32, tag="p2")
                for r0, nr in _row_chunks(h0, h0 + R):
                    _conv_bankset(
                        nc,
                        ps2.tile([128, 1024], F32, tag="ps2", name="pt2"),
                        w2, id2, s2_t,
                        soff=lambda t, _r=r0: 2
                        + (ploc(_r) + t // 3 - 1) * WP
                        + (t % 3 - 1),
                        res_t=p1_t,
                        roff=2 + ploc(r0) * WP,
                        n=nr * WP,
                        asc=as2,
                        bias_mid=b32,
                        alpha=v["a2"],
                        out_ap=p2_t[
                            :, (r0 - h0) * WP : (r0 - h0 + nr) * WP
                        ],
                        ev=evpool.tile([128, 456], F32, tag="ev", name="ev"),
                    )

                # -- out2 = p2 + b23, store -----------------------------
                nc.vector.tensor_scalar_add(p2_t, p2_t, v["b23"])
                p2_r = p2_t.rearrange("p (r c) -> p r c", c=WP)
                for j in range(2):
                    nc.sync.dma_start(
                        out=out_d[imgs[j], :, h0 : h0 + R, :],
                        in_=p2_r[64 * j : 64 * j + 64, :, 1 : 1 + W],
                    )


_NC_CACHE = {}


def _get_program(bl=BL):
    if bl not in _NC_CACHE:
        _NC_CACHE[bl] = build_program(bl)
    return _NC_CACHE[bl]


def make_in_maps(inputs):
    x = np.ascontiguousarray(np.asarray(inputs["x"], dtype=np.float32))
    shared = {
        "w3": np.ascontiguousarray(np.asarray(inputs["w3"], np.float32)),
        "w_pw": np.ascontiguousarray(np.asarray(inputs["w_pw"], np.float32)),
    }
    for n in WVEC_NAMES:
        shared[n] = np.ascontiguousarray(np.asarray(inputs[n], np.float32))
    return [{"x": x[i * BL : (i + 1) * BL], **shared} for i in range(NCORES)]


def run(inputs, trace=False, **kwargs):
    nc = _get_program(BL)
    res = run_bass_kernel_spmd(
        nc, make_in_maps(inputs), core_ids=list(range(NCORES)), trace=trace,
        **kwargs,
    )
    out = np.concatenate([r["out"] for r in res.results], axis=0)
    return out, res


def kernel(**inputs):
    return run(inputs)[0]


def bench(inputs, iters=20):
    """Steady-state wall-clock benchmark: sharded jit without donation,
    device-resident inputs, async dispatch of `iters` executions."""
    import time
    import jax
    from jax.sharding import Mesh, PartitionSpec, NamedSharding
    from jax.experimental.shard_map import shard_map
    from concourse import bass2jax as b2j

    b2j.install_neuronx_cc_hook()
    nc = _get_program(BL)
    in_maps = make_in_maps(inputs)

    in_names, out_names, out_avals = [], [], []
    for alloc in nc.m.functions[0].allocations:
        if not isinstance(mybir.MemoryLocationSet, type) or not isinstance(
            alloc, mybir.MemoryLocationSet
        ):
            continue
        name = alloc.memorylocations[0].name
        if alloc.kind == "ExternalInput":
            if nc.partition_id_tensor and name == nc.partition_id_tensor.name:
                continue
            in_names.append(name)
        elif alloc.kind == "ExternalOutput":
            out_names.append(name)
            out_avals.append(
                jax.core.ShapedArray(
                    tuple(alloc.tensor_shape), mybir.dt.np(alloc.dtype)
                )
            )
    n_params = len(in_names)
    all_names = in_names + out_names
    if nc.partition_id_tensor:
        all_names = all_names + [nc.partition_id_tensor.name]

    def _body(*args):
        operands = list(args)
        if nc.partition_id_tensor:
            operands.append(b2j.partition_id_tensor())
        outs = b2j._bass_exec_p.bind(
            *operands,
            out_avals=tuple(out_avals),
            in_names=tuple(all_names),
            out_names=tuple(out_names),
            lowering_input_output_aliases=(),
            sim_require_finite=True,
            sim_require_nnan=True,
            nc=nc,
        )
        return tuple(outs)

    devices = jax.devices()[:NCORES]
    mesh = Mesh(np.asarray(devices), ("core",))
    nin = n_params + len(out_names)
    f = jax.jit(
        shard_map(
            _body,
            mesh=mesh,
            in_specs=(PartitionSpec("core"),) * nin,
            out_specs=(PartitionSpec("core"),) * len(out_names),
            check_rep=False,
        ),
        keep_unused=True,
    )
    sh = NamedSharding(mesh, PartitionSpec("core"))
    concat_in = [
        jax.device_put(np.concatenate([m[n] for m in in_maps], axis=0), sh)
        for n in in_names
    ]
    zeros = [
        jax.device_put(
            np.zeros((NCORES * a.shape[0], *a.shape[1:]), a.dtype), sh
        )
        for a in out_avals
    ]

    r = f(*concat_in, *zeros)  # warm-up / compile
    jax.block_until_ready(r)

    res = {}
    for ntest in (1, 10, 30):
        ts = []
        for _ in range(3):
            t0 = time.perf_counter()
            rs = [f(*concat_in, *zeros) for _ in range(ntest)]
            jax.block_until_ready(rs)
            ts.append((time.perf_counter() - t0) / ntest)
        res[ntest] = min(ts)
    res["single_s"] = res[1]
    # slope between 10 and 30 removes the one-time dispatch ramp
    res["per_iter_s"] = (res[30] * 30 - res[10] * 10) / 20
    return res


if __name__ == "__main__":
    rng = np.random.default_rng(0)
    ins = {"x": rng.standard_normal((B, C, H, W)).astype(np.float32)}
    for n in ["w3", "w_pw"]:
        ins[n] = ((rng.random((C, C, 3, 3)) - 0.5) * 0.002).astype(np.float32)
    for n in WVEC_NAMES:
        ins[n] = (rng.standard_normal(C) * 0.01).astype(np.float32)
    out = kernel(**ins)
    print(out.shape, out.dtype)
